# revision 16
# baseline (speedup 1.0000x reference)
"""GCL layer (linear + sparse-Laplacian SpMM) on 8 TRN2 NeuronCores.

Algorithm:  out = L @ (X @ W.T + b)  ==  (L @ X) @ W.T + (L @ 1) b^T
The gather/segment-sum runs on raw features; the dense linear is applied
once per output row after aggregation.  Features are replicated to every
core's HBM at staging time (fp16); destination rows are sharded
contiguously across the 8 cores (12500 each), so the segment sum is
device-local.  No collectives are needed: staging already gives every
core the full feature table.

Per core, edges are sorted by (superblock, source-range, dest).  A
"block" is 512 destination rows, accumulated as Y^T [128 feat, 512 dest]
in one PSUM bank; a superblock is 4 blocks (4 live PSUM banks).  Chunks
of 128 edges are gathered with dma_gather (row per partition, fp16 =
256B/row); for each chunk a windowed one-hot scatter matrix
S[e, d] = (d == dest_e - dmin) * val_e, PRECOMPUTED host-side in fp16
and streamed via the ACT HWDGE queue (so it never touches the POOL/DVE
SBUF port that Q7 descriptor generation needs), feeds the PE:
    ybank[:, dmin:dmin+win] += gathered.T @ S        (fp16 matmul)
Per block afterwards: Y^T -> SBUF fp16 (ACT copy), four [K=128, M=128]
fp16 matmuls apply W^T (lhsT = Y^T directly), and the DVE stt
ot = bias*rowsum + po folds the bias add into the PSUM->SBUF move;
one 3D-AP DMA stores the block.

dma_gather limits/costs drive the layout:
  - int16 indices: sources split into 4 EQUAL ranges of 25000 rows
    (< 32768 int16 reach) so gather sizes are uniform.
  - descriptor generation runs ONLY on the Q7 core pair selected by
    queue_num (cpu 2q, 2q+1); each (superblock, range) region is split
    into four sub-gathers on the four queues (load-balanced), so all
    8 Q7 cores generate concurrently; dynamic_dma_scratch_size=49152
    gives the descriptor rings enough depth to decouple generation
    from SDMA drain.
  - the chunk->block/window schedule must be identical on all 8 cores
    (SPMD): segments are the UNION over cores of each chunk's
    (block, window); per-core S data zeroes the unused slots.
"""

import sys

for _p in ("/opt/trn_rl_repo",):
    if _p not in sys.path:
        sys.path.append(_p)

import numpy as np

# ---------------------------------------------------------------- constants
N_NODES = 100000
D = 128  # d_in == d_out == 128
N_CORES = 8
NPC = N_NODES // N_CORES  # 12500 destination rows per core
BLKW = 512  # destination rows per block (= one PSUM bank of Y^T)
SBB = 4  # blocks per superblock (PSUM banks live at once)
RANGE_ROWS = 25000  # equal ranges; < 32768 int16 gather reach
CHUNK = 128  # edges per matmul (PE contraction dim)
SW_COLS = 4096  # max S-stream group width (fp16 cols)


def _cdiv(a, b):
    return (a + b - 1) // b


# ---------------------------------------------------------------- host plan
def _plan(edge_rows, edge_cols, edge_vals):
    E = edge_rows.shape[0]
    nblocks = _cdiv(NPC, BLKW)
    nsb = _cdiv(nblocks, SBB)
    nranges = _cdiv(N_NODES, RANGE_ROWS)
    nregions = nsb * nranges

    rows = edge_rows.astype(np.int64)
    cols = edge_cols.astype(np.int64)
    vals = edge_vals.astype(np.float32)

    core = rows // NPC
    local = rows - core * NPC
    blk = local // BLKW
    rng = cols // RANGE_ROWS
    colloc = (cols - rng * RANGE_ROWS).astype(np.int16)

    sb = blk // SBB
    reg = sb * nranges + rng
    gkey = core * nregions + reg

    # sort by (core, region, dest) so windows are tight and blocks ordered
    order = np.lexsort((local, gkey))
    counts = np.bincount(gkey, minlength=N_CORES * nregions).reshape(
        N_CORES, nregions
    )
    # pad every (core, region) group with idx=0 / val=0 up to the uniform
    # region length (row 0 is fetched but contributes nothing); padding is
    # only ~2.5% of E at region granularity
    nmax = counts.max(axis=0)
    L = ((nmax + CHUNK - 1) // CHUNK) * CHUNK
    offs = np.zeros(nregions + 1, np.int64)
    offs[1:] = np.cumsum(L)
    total = int(offs[-1])

    gc = np.bincount(gkey, minlength=N_CORES * nregions)
    gstarts = np.concatenate([[0], np.cumsum(gc)[:-1]])
    rank = np.arange(E, dtype=np.int64) - gstarts[gkey[order]]
    pos = offs[reg[order]] + rank  # slot within the per-core layout
    ce = core[order]

    rowsum = np.bincount(rows, weights=vals.astype(np.float64), minlength=N_NODES)
    rowsum = rowsum.astype(np.float32)

    # ---- union segmentation: chunk x block-in-sb -> dest window
    nchunks = total // CHUNK
    dloc_all = (local - blk * BLKW).astype(np.int64)  # dest local to block
    ch = pos // CHUNK
    bin_sb = blk[order] - (blk[order] // SBB) * SBB
    k = ch * SBB + bin_sb  # (chunk, block-in-sb) cell
    ncells = nchunks * SBB
    cellcnt = np.bincount(k, minlength=ncells)
    dmin = np.full(ncells, 1 << 30, np.int64)
    dmax = np.full(ncells, -1, np.int64)
    np.minimum.at(dmin, k, dloc_all[order])
    np.maximum.at(dmax, k, dloc_all[order])
    cells = np.nonzero(cellcnt)[0]
    nseg = len(cells)
    segid_of_cell = np.full(ncells, -1, np.int64)
    segid_of_cell[cells] = np.arange(nseg)
    # per-segment tables (uniform across cores)
    seg_chunk = cells // SBB
    seg_b = cells % SBB  # block index within its superblock
    seg_dmin = dmin[cells]
    seg_win = (dmax[cells] - dmin[cells] + 1).astype(np.int64)

    # the first matmul into each PSUM bank must write the FULL bank (the
    # start bit zero-fills the whole 2KiB region; later windowed
    # accumulates must only touch already-written bytes)
    chunk_region = np.zeros(nchunks, np.int64)
    for r in range(nregions):
        chunk_region[offs[r] // CHUNK : offs[r + 1] // CHUNK] = r
    seg_block = (chunk_region[seg_chunk] // nranges) * SBB + seg_b
    first_seen = set()
    for i in range(nseg):  # cells are in emission order (chunk-major)
        b = int(seg_block[i])
        if b not in first_seen:
            first_seen.add(b)
            seg_dmin[i] = 0
            seg_win[i] = BLKW

    # segment offsets into the streamed S matrix (concatenated windows)
    seg_off = np.zeros(nseg + 1, np.int64)
    seg_off[1:] = np.cumsum(seg_win)
    sumwin = int(seg_off[-1])
    # split each region's segments into stream groups of <= SW_COLS columns
    reg_first_seg = np.searchsorted(seg_chunk, offs[:-1] // CHUNK)
    reg_last_seg = np.searchsorted(seg_chunk, offs[1:] // CHUNK)
    groups_by_reg = []  # per region: list of (soff, width, seg_lo, seg_hi)
    seg_group_off = np.zeros(nseg, np.int64)  # S col offset of seg's group
    for rg in range(nregions):
        lo = int(reg_first_seg[rg])
        hi = int(reg_last_seg[rg])
        glist = []
        i = lo
        while i < hi:
            j = i
            base = int(seg_off[i])
            while j < hi and int(seg_off[j + 1]) - base <= SW_COLS:
                j += 1
            glist.append((base, int(seg_off[j]) - base, i, j))
            seg_group_off[i:j] = base
            i = j
        groups_by_reg.append(glist)

    # ---- per-core staged arrays (S is PRECOMPUTED host-side in fp16)
    seg_of_edge = segid_of_cell[k]
    slot_in_chunk = pos % CHUNK
    t16 = total // 16
    percore = []
    for c in range(N_CORES):
        m = ce == c
        idx = np.zeros(total, np.int16)
        p = pos[m]
        o = order[m]
        idx[p] = colloc[o]
        s = seg_of_edge[m]
        smat = np.zeros((CHUNK, sumwin), np.float16)
        smat[
            slot_in_chunk[m],
            seg_off[s] + dloc_all[order][m] - seg_dmin[s],
        ] = vals[o].astype(np.float16)
        idx_w = np.tile(idx.reshape(t16, 16).T, (8, 1))  # [128, t16]
        # rowsum laid out [128, nblocks*4]: col b*4+j holds rowsums of
        # dest rows b*512 + j*128 + p (per-partition scalars for the bias stt)
        rs = np.pad(
            rowsum[c * NPC : (c + 1) * NPC], (0, nblocks * BLKW - NPC)
        ).astype(np.float32)
        rs = np.ascontiguousarray(rs.reshape(nblocks * (BLKW // 128), 128).T)
        percore.append(
            dict(
                idx=np.ascontiguousarray(idx_w),
                smat=np.ascontiguousarray(smat),
                rowsum=rs,
            )
        )

    sched = dict(
        nblocks=nblocks,
        nsb=nsb,
        nranges=nranges,
        nregions=nregions,
        L=L,
        offs=offs,
        total=total,
        nseg=nseg,
        seg_chunk=seg_chunk,
        seg_b=seg_b,
        seg_dmin=seg_dmin,
        seg_win=seg_win,
        seg_off=seg_off,
        sumwin=sumwin,
        groups_by_reg=groups_by_reg,
        seg_group_off=seg_group_off,
    )
    return sched, percore


# ---------------------------------------------------------------- device prog
def _build(sched):
    import concourse.bacc as bacc
    import concourse.mybir as mybir
    import concourse.tile as tile
    from contextlib import ExitStack
    from concourse.library_config import mlp

    f32 = mybir.dt.float32
    f16 = mybir.dt.float16
    i16 = mybir.dt.int16

    nblocks = sched["nblocks"]
    nsb = sched["nsb"]
    nranges = sched["nranges"]
    nregions = sched["nregions"]
    L = sched["L"]
    offs = sched["offs"]
    total = sched["total"]
    nseg = sched["nseg"]
    seg_chunk = sched["seg_chunk"]
    seg_b = sched["seg_b"]
    seg_dmin = sched["seg_dmin"]
    seg_win = sched["seg_win"]
    seg_off = sched["seg_off"]
    sumwin = sched["sumwin"]
    groups_by_reg = sched["groups_by_reg"]
    seg_group_off = sched["seg_group_off"]

    # group segments by chunk for the emit loop
    segs_by_chunk = {}
    for i in range(nseg):
        segs_by_chunk.setdefault(int(seg_chunk[i]), []).append(i)
    # per-block totals for PSUM start/stop flags
    blk_tot = np.zeros(nblocks, np.int64)
    chunk_region = np.zeros(total // CHUNK, np.int64)
    for r in range(nregions):
        chunk_region[offs[r] // CHUNK : offs[r + 1] // CHUNK] = r
    for i in range(nseg):
        r = chunk_region[seg_chunk[i]]
        b = (r // nranges) * SBB + int(seg_b[i])
        blk_tot[b] += 1

    nc = bacc.Bacc(
        "TRN2",
        target_bir_lowering=False,
        debug=False,
        num_devices=N_CORES,
        num_swdge_queues=4,
        dynamic_dma_scratch_size=49152,
    )

    feat = nc.dram_tensor("features", [N_NODES, D], f16, kind="ExternalInput")
    wt_d = nc.dram_tensor("wt", [D, D], f16, kind="ExternalInput")
    bias_d = nc.dram_tensor("bias_r", [128, D], f32, kind="ExternalInput")
    rowsum_d = nc.dram_tensor(
        "rowsum", [128, nblocks * (BLKW // 128)], f32, kind="ExternalInput"
    )
    smat_d = nc.dram_tensor("smat", [CHUNK, sumwin], f16, kind="ExternalInput")
    idx_d = nc.dram_tensor("idx", [128, total // 16], i16, kind="ExternalInput")
    out_d = nc.dram_tensor("out", [NPC, D], f32, kind="ExternalOutput")

    with tile.TileContext(nc) as tc, ExitStack() as ctx:
        const = ctx.enter_context(tc.tile_pool(name="const", bufs=1))
        gpool = ctx.enter_context(tc.tile_pool(name="gath", bufs=4))
        spool = ctx.enter_context(tc.tile_pool(name="smat", bufs=3))
        ypool = ctx.enter_context(tc.tile_pool(name="ysb", bufs=3))
        opool = ctx.enter_context(tc.tile_pool(name="osb", bufs=3))
        ypsum = ctx.enter_context(tc.tile_pool(name="ypsum", bufs=5, space="PSUM"))
        opsum = ctx.enter_context(tc.tile_pool(name="opsum", bufs=2, space="PSUM"))

        nc.gpsimd.load_library(mlp)

        wt_t = const.tile([D, D], f16, tag="wt")
        nc.sync.dma_start(wt_t[:], wt_d.ap())
        bias_t = const.tile([128, D], f32, tag="bias")
        nc.sync.dma_start(bias_t[:], bias_d.ap())
        rowsum_t = const.tile([128, nblocks * (BLKW // 128)], f32, tag="rowsum")
        nc.sync.dma_start(rowsum_t[:], rowsum_d.ap())
        idx_t = const.tile([128, total // 16], i16, tag="idx")
        nc.sync.dma_start(idx_t[:], idx_d.ap())

        feat_ap = feat.ap()
        smat_ap = smat_d.ap()
        blk_seen = [0] * nblocks
        qload = [0, 0, 0, 0]  # per-queue descriptor-count balancing
        lmax = int(L.max())

        for s in range(nsb):
            blocks = [b for b in range(s * SBB, min((s + 1) * SBB, nblocks))]
            ybanks = {
                b: ypsum.tile([128, BLKW], f32, tag="yb", name="yb") for b in blocks
            }
            for r in range(nranges):
                rid = s * nranges + r
                n = int(L[rid])
                if n == 0:
                    continue

                o = int(offs[rid])
                g = gpool.tile([128, lmax // CHUNK, D], f16, tag="g", name="g")
                lo = r * RANGE_ROWS
                hi = min(N_NODES, lo + RANGE_ROWS)
                # split the region gather four ways, one sub-gather per SWDGE
                # queue: each queue number activates a different Q7 core pair,
                # so all four pairs generate this region's descriptors
                # concurrently
                nq = (n // 4 // CHUNK) * CHUNK
                cuts = [0, nq, 2 * nq, 3 * nq, n]
                for (c0, c1) in zip(cuts[:-1], cuts[1:]):
                    if c1 == c0:
                        continue
                    q = min(range(4), key=lambda i: qload[i])
                    qload[q] += c1 - c0
                    nc.gpsimd.dma_gather(
                        g[:, c0 // CHUNK : c1 // CHUNK, :],
                        feat_ap[lo:hi, :],
                        idx_t[:, (o + c0) // 16 : (o + c1) // 16],
                        c1 - c0,
                        c1 - c0,
                        D,
                        single_packet=False,
                        queue_num=q,
                    )
                # stream this region's precomputed S windows from HBM
                sg_tiles = {}
                for (soff, sw, slo, shi) in groups_by_reg[rid]:
                    st = spool.tile([128, SW_COLS], f16, tag="s", name="sreg")
                    nc.scalar.dma_start(st[:, :sw], smat_ap[:, soff : soff + sw])
                    for q in range(slo, shi):
                        sg_tiles[q] = st
                for t in range(n // CHUNK):
                    gch = o // CHUNK + t
                    for si in segs_by_chunk.get(gch, ()):
                        b = s * SBB + int(seg_b[si])
                        dmin = int(seg_dmin[si])
                        win = int(seg_win[si])
                        sl = int(seg_off[si]) - int(seg_group_off[si])
                        nc.tensor.matmul(
                            ybanks[b][:, dmin : dmin + win],
                            g[:, t, :],
                            sg_tiles[si][:, sl : sl + win],
                            start=(blk_seen[b] == 0),
                            stop=(blk_seen[b] == blk_tot[b] - 1),
                        )
                        blk_seen[b] += 1

            # drain superblock: linear per block, bias folded into the
            # PSUM->SBUF move (DVE stt: ot = bias*rowsum + po)
            for b in blocks:
                w = min(BLKW, NPC - b * BLKW)
                nsub = _cdiv(w, 128)
                ot = opool.tile([128, BLKW], f32, tag="o")
                if blk_tot[b] > 0:
                    yt = ypool.tile([128, BLKW], f16, tag="y")
                    nc.scalar.copy(yt[:, : nsub * 128], ybanks[b][:, : nsub * 128])
                    po = opsum.tile([128, BLKW], f32, tag="po")
                    for j in range(nsub):
                        nc.tensor.matmul(
                            po[:, j * 128 : j * 128 + D],
                            yt[:, j * 128 : j * 128 + 128],
                            wt_t[:],
                            start=(j == 0),
                            stop=(j == nsub - 1),
                        )
                    for j in range(nsub):
                        nc.vector.scalar_tensor_tensor(
                            ot[:, j * 128 : j * 128 + D],
                            bias_t[:],
                            rowsum_t[
                                :,
                                b * (BLKW // 128) + j : b * (BLKW // 128) + j + 1,
                            ],
                            po[:, j * 128 : j * 128 + D],
                            op0=mybir.AluOpType.mult,
                            op1=mybir.AluOpType.add,
                        )
                else:
                    for j in range(nsub):
                        nc.vector.tensor_scalar(
                            ot[:, j * 128 : j * 128 + D],
                            bias_t[:],
                            rowsum_t[
                                :,
                                b * (BLKW // 128) + j : b * (BLKW // 128) + j + 1,
                            ],
                            None,
                            op0=mybir.AluOpType.mult,
                        )
                # store: DRAM rows b*BLKW + j*128 + p  <-  ot[p, j*128 + d]
                if w == BLKW:
                    r0 = b * BLKW
                    oap = out_d.ap()[r0 : r0 + BLKW, :].rearrange(
                        "(j p) d -> p j d", p=128
                    )
                    nc.sync.dma_start(oap, ot[:, : nsub * 128])
                else:
                    for j in range(nsub):
                        wj = min(128, w - j * 128)
                        r0 = b * BLKW + j * 128
                        nc.sync.dma_start(
                            out_d.ap()[r0 : r0 + wj, :],
                            ot[:wj, j * 128 : j * 128 + D],
                        )

    nc.compile()
    return nc


def _stage_inputs(inputs, sched, percore):
    feat16 = np.ascontiguousarray(inputs["features"].astype(np.float16))
    wt = np.ascontiguousarray(inputs["weight"].astype(np.float16).T)
    bias_r = np.ascontiguousarray(
        np.tile(inputs["bias"].astype(np.float32).reshape(1, D), (128, 1))
    )
    in_maps = []
    for c in range(N_CORES):
        in_maps.append(
            dict(
                features=feat16,
                wt=wt,
                bias_r=bias_r,
                rowsum=percore[c]["rowsum"],
                smat=percore[c]["smat"],
                idx=percore[c]["idx"],
            )
        )
    return in_maps


# ---------------------------------------------------------------- entry point
def kernel(features, weight, bias, edge_vals, edge_rows, edge_cols):
    from concourse.bass_utils import run_bass_kernel_spmd

    sched, percore = _plan(edge_rows, edge_cols, edge_vals)
    nc = _build(sched)

    inputs = dict(features=features, weight=weight, bias=bias)
    in_maps = _stage_inputs(inputs, sched, percore)

    res = run_bass_kernel_spmd(nc, in_maps, core_ids=list(range(N_CORES)))
    out = np.concatenate([res.results[c]["out"] for c in range(N_CORES)], axis=0)
    return out
